# revision 22
# baseline (speedup 1.0000x reference)
"""
LongConvolution (causal FFT conv) Trainium2 Bass kernel — v4.

Problem: x (4, 8192, 1024) f32, filt (1024, 8192) f32.
  y[b, l, c] = sum_m x[b, m, c] * filt[c, l - m]   (causal, per-channel)
Reference computes this via zero-padded FFT of size N = 16384 = 128*128.

v4 over v3: the three pointwise complex-multiply stages (T1 twiddle,
filter PW, T2 conj-twiddle) run as ONE custom DVE instruction each,
using a hand-written uop program (CMUL_I_ANT, registered at runtime
into dve_ops) that processes one interleaved (re,im) f16 pair per
cycle in 2X_1PORT mode — ~3x fewer DVE cycles than the stock
tensor_tensor decomposition.  That frees the DVE, so:
  - PSUM->SBUF converting copies interleave re/im pairs on the READ
    side (strided PSUM reads are cheap; strided SBUF writes and
    strided matmul PSUM writes both measured expensive) and are
    load-balanced across ACT and DVE (GpSimd has no PSUM port);
  - I2 drops from 4 to 2 accumulation passes (explicit cp from the
    fused T2) — PE 28672 -> 24576 cycles/iteration.
Measured: 322us (v3) -> 240.4us best (noise band ~240-247us;
reset cores before benchmarking), rel err 6.3e-4.

Four-step FFT (k = 128*k2 + k1, n = 128*n1 + n2):
  A[n2,k1] = sum_n1 e^{-2pi i n1 k1/128} z[128 n1 + n2]     (F1, K=64x2)
  B = A * T1,  T1[n2,k1] = e^{-2pi i n2 k1/N}               (T1, CMUL)
  R[k2,k1] = sum_n2 e^{-2pi i n2 k2/128} B[n2,k1]           (F2)
  P = R * K   (filter spectrum, host-precomputed, [k2,k1])  (PW, CMUL)
  C[k1,n2] = sum_k2 e^{+2pi i n2 k2/128} P[k2,k1]           (I1)
  cp = C * conj(T1)  ([k1,n2] layout, T1 symmetric)         (T2, CMUL)
  y[n1,n2] = sum_k1 e^{+2pi i n1 k1/128} cp[k1,n2], n1<64   (I2, 2-pass)
  z = x[2p] + i*x[2p+1];  y[2p] = Re, y[2p+1] = Im.

Sharding: d_model across the 8 cores (128 channels each).
"""

import os
import sys

import numpy as np

for p in ("/opt/trn_rl_repo",):
    if p not in sys.path:
        sys.path.insert(0, p)

os.environ.setdefault("MYCRO_LOCAL_CACHE", "1")

# ----------------------------------------------------------------------------
# configuration
# ----------------------------------------------------------------------------
B, L, D = 4, 8192, 1024
NFFT = 2 * L               # 16384 = 128 * 128
NC = 8                     # cores
CPC = D // NC              # channels per core = 128
CHG = 8                    # channels per iteration
NIT = CPC // CHG           # 16 iterations


# ----------------------------------------------------------------------------
# custom DVE op: interleaved complex multiply, 1 complex / cycle (2X_1PORT)
# ----------------------------------------------------------------------------
def _build_pair_uop():
    from concourse.dve_uop import (
        ENABLE,
        AluInp,
        AluOp,
        DelayInp,
        InpSel,
        OutPath,
        OutSel,
        Trigger,
        UopConfig,
    )

    u = UopConfig()
    u.enable_input(InpSel.SRC_0, 1)      # lane1 -> PREV_DELAY_0 @ blk0 (a_re)
    u.enable_input(InpSel.SRC_0_HI, 2)   # lane2 -> PREV_DELAY_1 (a_im)
    u.enable_input(InpSel.SRC_1, 3)      # lane3 -> PREV_DELAY_2 (t_re)
    u.enable_input(InpSel.SRC_1_HI, 4)   # lane4 -> PREV_DELAY_3 (t_im)
    u.require_inp0 = ENABLE
    u.require_inp1 = ENABLE
    u.trigger = (Trigger.SRC_TENSOR_DONE, Trigger.NONE, Trigger.NONE)
    u.next_uop = (0, 0, 0)

    b = u.datapath_config
    b[0].enable_alu(AluOp.MULTIPLY, AluInp.PREV_DELAY_0, AluInp.PREV_DELAY_2)
    b[0].pass_through_delay(0, 1, 2, 3)
    b[1].enable_alu(AluOp.MULTIPLY, AluInp.PREV_DELAY_1, AluInp.PREV_DELAY_3)
    b[1].pass_through_delay(0, 1, 2, 3)
    b[1].enable_delay_from_src(DelayInp.PREV_ALU_OUT, 4)
    b[2].enable_alu(AluOp.SUBTRACT, AluInp.PREV_DELAY_4, AluInp.PREV_ALU_OUT)
    b[2].pass_through_delay(0, 1, 2, 3)
    b[3].enable_alu(AluOp.MULTIPLY, AluInp.PREV_DELAY_0, AluInp.PREV_DELAY_3)
    b[3].pass_through_delay(1, 2)
    b[3].enable_delay_from_src(DelayInp.PREV_ALU_OUT, 5)
    b[4].enable_alu(AluOp.MULTIPLY, AluInp.PREV_DELAY_1, AluInp.PREV_DELAY_2)
    b[4].pass_through_delay(5)
    b[4].enable_delay_from_src(DelayInp.PREV_ALU_OUT, 4)
    b[5].enable_alu(AluOp.ADD, AluInp.PREV_DELAY_4, AluInp.PREV_ALU_OUT)
    b[5].pass_through_delay(5)
    b[6].pass_through_alu()
    b[6].pass_through_delay(5)
    b[7].pass_through_alu()
    b[7].pass_through_delay(5)

    u.enable_output(OutSel.DELAY_5, OutPath.WR0_LO)   # b_re
    u.enable_output(OutSel.ALU_OUT, OutPath.WR0_HI)   # b_im
    return u


def _build_sentinel_uop():
    """REGULAR-slot sentinel (out = MAX_POS).  Our call sites always satisfy
    the 2X_1PORT conditions (f16, packed step-1, 4B aligned), so this slot
    never runs; if it ever did, validation would fail loudly."""
    from concourse.dve_uop import (
        ENABLE,
        AluInp,
        AluOp,
        InpSel,
        OutPath,
        OutSel,
        Trigger,
        UopConfig,
    )

    u = UopConfig()
    u.enable_input(InpSel.SRC_0, 1)
    u.enable_input(InpSel.MAX_POS, 2)
    u.require_inp0 = ENABLE
    u.require_inp1 = ENABLE
    u.trigger = (Trigger.SRC_TENSOR_DONE, Trigger.NONE, Trigger.NONE)
    b = u.datapath_config
    b[0].enable_alu(AluOp.BYPASS, AluInp.PREV_DELAY_1, AluInp.PREV_DELAY_1)
    for k in range(1, 8):
        b[k].pass_through_alu()
    u.enable_output(OutSel.ALU_OUT, OutPath.WR0_LO)
    return u


def _cmul_ref(in0, in1, c0, c1, c2):
    pdim = np.asarray(in0).shape[0]
    a = np.asarray(in0, np.float32).reshape(pdim, -1)
    t = np.ascontiguousarray(np.asarray(in1, np.float32)).reshape(pdim, -1)
    if t.shape != a.shape:
        t = np.broadcast_to(t, a.shape)
    out = np.empty_like(a)
    ar, ai = a[..., 0::2], a[..., 1::2]
    tr, ti = t[..., 0::2], t[..., 1::2]
    out[..., 0::2] = ar * tr - ai * ti
    out[..., 1::2] = ar * ti + ai * tr
    return out.reshape(np.asarray(in0).shape)


def _register_cmul():
    from concourse import dve_ops
    from concourse.dve_spec import Spec, Src0, Src1
    from concourse.dve_uop import DveOpSpec

    name = "CMUL_I_ANT"
    for op in dve_ops.OPS:
        if op.name == name:
            return op
    spec = Spec(body=Src0 * Src1, reference=_cmul_ref)
    op = dve_ops.DveOp(name, spec, subdim=False, uops_sha={})
    dve_ops.OPS.append(op)
    dve_ops._SUB_OPCODE_FOR_NAME[name] = (
        dve_ops._CUSTOM_DVE_ROW_BASE + len(dve_ops.OPS) - 1
    )
    dve_ops.CUSTOM_DVE_SPECS[name] = spec
    compiled = DveOpSpec(
        name=name,
        opcode=dve_ops.get_dve_sub_opcode(name),
        uops=[_build_sentinel_uop()],
        uops_2x=[_build_pair_uop()],
        perf_max=1,
        rd1_en=True,
    )
    compiled.validate("v3")
    dve_ops._COMPILE_CACHE[(name, "v3")] = compiled
    return op


def _emit_cmul(nc, out, in0, in1):
    """nc.vector._custom_dve clone that sets perf_max=1 on the instruction."""
    import concourse.mybir as mybir
    from concourse import bass_isa
    from concourse.dve_ops import get_dve_sub_opcode

    v = nc.vector
    op = _register_cmul()
    if op.name not in v.bass.m.ant_custom_dve_ops:
        v.bass.m.ant_custom_dve_ops = sorted(
            {*v.bass.m.ant_custom_dve_ops, op.name}
        )
    in1_elementwise = len(in1.shape) > 2
    shape = (
        bass_isa.CustomDveShape.STT
        if in1_elementwise
        else bass_isa.CustomDveShape.TTSS
    )
    isa_opcode = v.bass.isa.Opcode[
        f"NEURON_ISA_TPB_OPCODE_CUSTOM_DVE_ANT_{shape.slot()}"
    ].value
    zero = mybir.ImmediateValue(dtype=mybir.dt.float32, value=0.0)
    zero2 = mybir.ImmediateValue(dtype=mybir.dt.float32, value=0.0)
    ins = [
        v.lower_ap(in0, for_isa=True, opt=True),
        v.lower_ap(in1, for_isa=True, opt=True),
        zero,
        zero2,
    ]
    outs = [v.lower_ap(out, for_isa=True, opt=True)]
    return v.add_instruction(
        bass_isa.InstCustomDveAnt(
            name=v.bass.get_next_instruction_name(),
            op_name=op.name,
            rd1_en=True,
            subdim=0,
            imm2=0.0,
            shape=shape,
            row=get_dve_sub_opcode(op.name),
            isa_opcode=isa_opcode,
            perf_max=1,
            ins=ins,
            outs=outs,
        )
    )


# ----------------------------------------------------------------------------
# host constants
# ----------------------------------------------------------------------------
def _consts():
    j = np.arange(128)
    ang128 = 2 * np.pi * np.outer(j, j) / 128
    angN = 2 * np.pi * np.outer(j, j) / NFFT
    return {
        "F_cos": np.cos(ang128), "F_sin": np.sin(ang128),
        "Tw_cos": np.cos(angN), "Tw_sin": np.sin(angN),
    }


def _interleave(re, im):
    """[.., n] x2 -> [.., 2n] with (re, im) pairs."""
    out = np.empty(re.shape[:-1] + (2 * re.shape[-1],), re.dtype)
    out[..., 0::2] = re
    out[..., 1::2] = im
    return out


def _host_arrays():
    cst = _consts()
    F_cos, F_sin = cst["F_cos"], cst["F_sin"]
    Tw_cos, Tw_sin = cst["Tw_cos"], cst["Tw_sin"]
    f16 = np.float16
    cosF, sinF = F_cos[:64, :], F_sin[:64, :]
    arrs = {}
    # stacked F1 moving: rows 0:64 act on z_re, rows 64:128 on z_im
    arrs["f1m"] = np.block([[cosF, -sinF], [sinF, cosF]]).astype(f16)
    arrs["f2c"] = F_cos.astype(f16)
    arrs["f2s"] = F_sin.astype(f16)
    arrs["f2sn"] = (-F_sin).astype(f16)
    arrs["fim"] = np.concatenate(
        [F_cos, F_sin, -F_sin, F_cos], axis=1
    ).astype(f16)
    # stacked I2 stationaries: out partitions 0:64 = Re (y even batch),
    # 64:128 = Im (y odd batch)
    arrs["gcs"] = np.concatenate(
        [F_cos[:, :64], F_sin[:, :64]], axis=1
    ).astype(f16)
    arrs["gnc"] = np.concatenate(
        [-F_sin[:, :64], F_cos[:, :64]], axis=1
    ).astype(f16)
    # interleaved twiddles: t1 = e^{-2pi i pq/N}, t2 = conj(t1)
    arrs["t1i"] = _interleave(Tw_cos, -Tw_sin).astype(f16)
    arrs["t2i"] = _interleave(Tw_cos, Tw_sin).astype(f16)
    return arrs


def _build_program():
    import concourse.bacc as bacc
    import concourse.mybir as mybir
    from concourse import tile

    f32 = mybir.dt.float32
    f16 = mybir.dt.float16

    _register_cmul()
    nc = bacc.Bacc(None, target_bir_lowering=False, debug=False)

    # --- DRAM I/O (all f16) ---
    # xw[it, (ri,n1), 2*ch+pk, n2] — stacked z_re/z_im chunks
    xw = nc.dram_tensor("xw", (NIT, 128, 2 * CHG, 128), f16, kind="ExternalInput")
    # kk[it, k2, ch, (k1,ri) interleaved]
    kk = nc.dram_tensor("kk", (NIT, 128, CHG, 256), f16, kind="ExternalInput")
    f1m_d = nc.dram_tensor("f1m", (128, 256), f16, kind="ExternalInput")
    f2c_d = nc.dram_tensor("f2c", (128, 128), f16, kind="ExternalInput")
    f2s_d = nc.dram_tensor("f2s", (128, 128), f16, kind="ExternalInput")
    f2sn_d = nc.dram_tensor("f2sn", (128, 128), f16, kind="ExternalInput")
    fim_d = nc.dram_tensor("fim", (128, 512), f16, kind="ExternalInput")
    gcs_d = nc.dram_tensor("gcs", (128, 128), f16, kind="ExternalInput")
    gnc_d = nc.dram_tensor("gnc", (128, 128), f16, kind="ExternalInput")
    t1i_d = nc.dram_tensor("t1i", (128, 256), f16, kind="ExternalInput")
    t2i_d = nc.dram_tensor("t2i", (128, 256), f16, kind="ExternalInput")
    # yw[pair, cl, (sig,n1), pk, n2] — two channels share one PSUM bank
    yw = nc.dram_tensor(
        "yw", (CPC // 2, 2, 128, 2, 128), f16, kind="ExternalOutput"
    )

    with tile.TileContext(nc) as tc:
        with (
            tc.tile_pool(name="const", bufs=1) as constp,
            tc.tile_pool(name="m", bufs=2) as mp,
            tc.tile_pool(name="kf", bufs=2) as kp,
            tc.tile_pool(name="work", bufs=3) as wp,
            tc.tile_pool(name="out", bufs=3) as op,
            tc.tile_pool(name="pa", bufs=2, space="PSUM") as pap,
            tc.tile_pool(name="pr", bufs=2, space="PSUM") as prp,
            tc.tile_pool(name="pc", bufs=2, space="PSUM") as pcp,
            tc.tile_pool(name="py", bufs=2, space="PSUM") as pyp,
        ):
            f1m = constp.tile([128, 256], f16)
            f2c = constp.tile([128, 128], f16)
            f2s = constp.tile([128, 128], f16)
            f2sn = constp.tile([128, 128], f16)
            fim = constp.tile([128, 512], f16)
            gcs = constp.tile([128, 128], f16)
            gnc = constp.tile([128, 128], f16)
            t1i = constp.tile([128, 256], f16)
            t2i = constp.tile([128, 256], f16)
            nc.sync.dma_start(f1m[:], f1m_d[:])
            # broadcast views over the 16 (ch,pk) signals of an iteration
            t1i_b = (
                t1i[:].rearrange("p (s n) -> p s n", s=1)
                .broadcast_to([128, CHG, 256])
            )
            t2i_b = (
                t2i[:].rearrange("p (s n) -> p s n", s=1)
                .broadcast_to([128, CHG, 256])
            )

            st = {}  # it -> dict of live tiles

            def e_dma(it):
                s = st[it] = {}
                s["m4"] = mp.tile([128, 2 * CHG, 128], f16, tag="m", name="m4")
                nc.sync.dma_start(s["m4"][:], xw[it])
                s["kt"] = kp.tile([128, CHG, 256], f16, tag="k", name="kt")
                nc.sync.dma_start(s["kt"][:], kk[it])

            def e_f1(it):
                # asrc[n2, j, (k1,ri)] <- contiguous copies of interleaved pa
                s = st[it]
                s["asrc"] = wp.tile([128, 2 * CHG, 256], f16, tag="asrc", name="asrc")
                for ch in range(CHG):
                    pa = pap.tile([128, 2, 2, 128], f32, tag="pa")
                    for pk in range(2):
                        nc.tensor.matmul(
                            pa[:, :, pk, :], s["m4"][:, 2 * ch + pk, :],
                            f1m[:], start=True, stop=True,
                        )
                    # interleave on the READ side: (ri,pk,k1) -> (pk,k1,ri)
                    nc.scalar.copy(
                        out=s["asrc"][:, 2 * ch : 2 * ch + 2, :],
                        in_=pa[:].rearrange("p r q k -> p q k r"),
                    )

            def e_t1(it):
                s = st[it]
                s["bt"] = wp.tile([128, 2 * CHG, 256], f16, tag="bt", name="bt")
                _emit_cmul(nc, s["bt"][:, :CHG], s["asrc"][:, :CHG], t1i_b)
                _emit_cmul(nc, s["bt"][:, CHG:], s["asrc"][:, CHG:], t1i_b)

            def e_f2(it):
                s = st[it]
                bv = s["bt"][:].rearrange("p j (k r) -> p j k r", r=2)
                # rsrc[k2, pk, ch, (k1,ri)]
                s["rsrc"] = wp.tile([128, 2, CHG, 256], f16, tag="rsrc", name="rsrc")
                for ch in range(CHG):
                    pr = prp.tile([128, 2, 2, 128], f32, tag="pr")
                    for pk in range(2):
                        j = 2 * ch + pk
                        b_re = bv[:, j, :, 0]
                        b_im = bv[:, j, :, 1]
                        nc.tensor.matmul(
                            pr[:, 0, pk, :], f2c[:], b_re,
                            start=True, stop=False,
                        )
                        nc.tensor.matmul(
                            pr[:, 0, pk, :], f2s[:], b_im,
                            start=False, stop=True,
                        )
                        nc.tensor.matmul(
                            pr[:, 1, pk, :], f2c[:], b_im,
                            start=True, stop=False,
                        )
                        nc.tensor.matmul(
                            pr[:, 1, pk, :], f2sn[:], b_re,
                            start=False, stop=True,
                        )
                    if ch < 4:
                        nc.scalar.copy(
                            out=s["rsrc"][:, :, ch, :],
                            in_=pr[:].rearrange("p r q k -> p q k r"),
                        )
                    else:
                        nc.vector.tensor_copy(
                            s["rsrc"][:, :, ch, :],
                            pr[:].rearrange("p r q k -> p q k r"),
                        )

            def e_pw(it):
                s = st[it]
                s["pt"] = wp.tile([128, 2, CHG, 256], f16, tag="pt", name="pt")
                for pk in range(2):
                    _emit_cmul(
                        nc, s["pt"][:, pk], s["rsrc"][:, pk], s["kt"][:]
                    )

            def e_i1(it):
                s = st[it]
                pv = s["pt"][:].rearrange("p q c (k r) -> p q c k r", r=2)
                s["csrc"] = wp.tile([128, 2 * CHG, 256], f16, tag="csrc", name="csrc")
                for ch in range(CHG):
                    pc = pcp.tile([128, 2, 2, 128], f32, tag="pc")
                    for pk in range(2):
                        p_re = pv[:, pk, ch, :, 0]
                        p_im = pv[:, pk, ch, :, 1]
                        nc.tensor.matmul(
                            pc[:, :, pk, :], p_re, fim[:, 0:256],
                            start=True, stop=False,
                        )
                        nc.tensor.matmul(
                            pc[:, :, pk, :], p_im, fim[:, 256:512],
                            start=False, stop=True,
                        )
                    iv = pc[:].rearrange("p r q k -> p q k r")
                    if ch < 5:
                        nc.scalar.copy(
                            out=s["csrc"][:, 2 * ch : 2 * ch + 2, :], in_=iv
                        )
                    else:
                        nc.vector.tensor_copy(
                            s["csrc"][:, 2 * ch : 2 * ch + 2, :], iv
                        )

            def e_t2(it):
                s = st[it]
                s["cpt"] = wp.tile([128, 2 * CHG, 256], f16, tag="cpt", name="cpt")
                _emit_cmul(nc, s["cpt"][:, :CHG], s["csrc"][:, :CHG], t2i_b)
                _emit_cmul(nc, s["cpt"][:, CHG:], s["csrc"][:, CHG:], t2i_b)

            def e_i2(it):
                s = st[it]
                cv = s["cpt"][:].rearrange("p j (k r) -> p j k r", r=2)
                py = None
                for ch in range(CHG):
                    cl = ch % 2
                    if cl == 0:
                        py = pyp.tile([128, 2, 2, 128], f32, tag="py")
                    for pk in range(2):
                        j = 2 * ch + pk
                        nc.tensor.matmul(
                            py[:, cl, pk, :], gcs[:], cv[:, j, :, 0],
                            start=True, stop=False,
                        )
                        nc.tensor.matmul(
                            py[:, cl, pk, :], gnc[:], cv[:, j, :, 1],
                            start=False, stop=True,
                        )
                    if cl == 1:
                        pair = (CHG * it + ch) // 2
                        ysb = op.tile([128, 2, 2, 128], f16, tag="ysb")
                        if ch in (1, 5):
                            nc.scalar.copy(out=ysb[:], in_=py[:])
                        else:
                            nc.vector.tensor_copy(ysb[:], py[:])
                        nc.sync.dma_start(
                            yw[pair].rearrange("c p k n -> p c k n"), ysb[:]
                        )
                del st[it]

            for pi in range(NIT // 2):
                e, o = 2 * pi, 2 * pi + 1
                e_dma(e)
                e_dma(o)
                if pi == 0:
                    # remaining consts AFTER the first pair's inputs so
                    # F1(0) is not starved behind serialized DMAs
                    for t, d in (
                        (t1i, t1i_d), (t2i, t2i_d), (f2c, f2c_d),
                        (f2s, f2s_d), (f2sn, f2sn_d), (fim, fim_d),
                        (gcs, gcs_d), (gnc, gnc_d),
                    ):
                        nc.sync.dma_start(t[:], d[:])
                e_f1(e)
                e_f1(o)
                e_t1(e)
                e_f2(e)
                e_t1(o)
                e_f2(o)
                e_pw(e)
                e_i1(e)
                e_pw(o)
                e_i1(o)
                e_t2(e)
                e_i2(e)
                e_t2(o)
                e_i2(o)

    nc.compile()
    return nc


def _prep_inputs(x, filt):
    """Full inputs -> list of per-core input maps."""
    consts = _host_arrays()

    kpad = np.zeros((D, NFFT), np.float64)
    kpad[:, :L] = filt
    Kf = (np.fft.fft(kpad, axis=1) / NFFT).reshape(D, 128, 128)  # [c, k2, k1]

    # x -> (D, 2pk, 2ri, 64 n1, 128 n2)
    xq = np.ascontiguousarray(x.transpose(2, 0, 1)).reshape(D, 2, 2, 64, 128)

    in_maps = []
    for ci in range(NC):
        sl = slice(ci * CPC, (ci + 1) * CPC)
        m = dict(consts)
        xc = xq[sl].reshape(NIT, CHG, 2, 2, 64, 128)
        # -> (it, (ri,n1), (chl,pk), n2)
        m["xw"] = np.ascontiguousarray(
            xc.transpose(0, 3, 4, 1, 2, 5).reshape(NIT, 128, 2 * CHG, 128)
        ).astype(np.float16)
        kc = Kf[sl].reshape(NIT, CHG, 128, 128)  # (it, ch, k2, k1)
        ki = _interleave(kc.real, kc.imag)       # (it, ch, k2, 256)
        m["kk"] = np.ascontiguousarray(
            ki.transpose(0, 2, 1, 3)             # (it, k2, ch, 256)
        ).astype(np.float16)
        in_maps.append(m)
    return in_maps


def _post_outputs(res):
    y = np.empty((B, L, D), np.float32)
    for ci in range(NC):
        sl = slice(ci * CPC, (ci + 1) * CPC)
        # (pair, cl, (sig,n1), pk, n2); c = 2*pair+cl, b = 2*pk+sig,
        # l = 128*n1+n2
        r = res.results[ci]["yw"].astype(np.float32)
        r = r.reshape(CPC // 2, 2, 2, 64, 2, 128)
        r = r.transpose(4, 2, 3, 5, 0, 1).reshape(B, L, CPC)
        y[:, :, sl] = r
    return y


def kernel(x: np.ndarray, filt: np.ndarray) -> np.ndarray:
    from concourse.bass_utils import run_bass_kernel_spmd

    assert x.shape == (B, L, D) and filt.shape == (D, L)
    x = np.ascontiguousarray(x, dtype=np.float32)
    filt = np.ascontiguousarray(filt, dtype=np.float32)

    in_maps = _prep_inputs(x, filt)
    nc = _build_program()
    res = run_bass_kernel_spmd(nc, in_maps, core_ids=list(range(NC)))
    return _post_outputs(res)


def run_profiled(inputs):
    """Build + run with NTFF tracing; returns BassKernelResults (test-only)."""
    from concourse.bass_utils import run_bass_kernel_spmd

    x = np.ascontiguousarray(inputs["x"], dtype=np.float32)
    filt = np.ascontiguousarray(inputs["filt"], dtype=np.float32)
    in_maps = _prep_inputs(x, filt)
    nc = _build_program()
    return run_bass_kernel_spmd(
        nc, in_maps, core_ids=list(range(NC)), trace=True
    )


if __name__ == "__main__":
    rng = np.random.default_rng(0)
    x = rng.standard_normal((B, L, D)).astype(np.float32)
    filt = rng.standard_normal((D, L)).astype(np.float32)
    y = kernel(x, filt)
    print("y", y.shape, y.dtype, float(np.abs(y).max()))
